# revision 1
# baseline (speedup 1.0000x reference)
"""Trainium2 Bass kernel v2 for nn_AttentionLayer (sparse graph attention + BN).

Strategy (8 cores, SPMD):
  - Host assigns nodes to 784 windows of 128 slots (serpentine deal by degree,
    plus a repair pass) so every window owns <= tpw*128 incident edges; 98
    windows per core. Output rows are produced in this "virtual" order and
    un-permuted on the host at the end.
  - For each edge tile (128 edges) the host pre-gathers transposed bf16
    operand panels into one blob: x[src] chunks (lhsT for K/V), x[dst] chunks
    (lhsT for Q), edge_attr chunks (lhsT for E), and the one-hot dst-slot
    matrix (lhsT for the segment-sum). No on-device gather or transpose.
  - Device per tile: 7 bf16 matmuls (KV, Q, EH projections + segment-sum),
    score pipeline on vector/scalar/gpsimd, PSUM-accumulated per window.
  - BatchNorm: per-core partial sums accumulated on the PE across all windows
    into one PSUM bank; host reduces across cores between the two launches;
    a second tiny kernel applies the affine transform.
"""

import math
import numpy as np
import ml_dtypes

import concourse.bass as bass
import concourse.tile as tile
from concourse import mybir
from concourse.bass_utils import run_bass_kernel_spmd

F32 = mybir.dt.float32
BF16 = mybir.dt.bfloat16

CORES = 8
N = 100000
E = 500000
DIM = 256
H = 8
DH = DIM // H
WPC = 98                  # windows per core
W = CORES * WPC           # 784 windows of 128 slots
NV = W * 128              # 100352 virtual node slots
EPS_Z = 1e-6
EPS_BN = 1e-5


# ----------------------------------------------------------------- host prep

def _balance_windows(deg):
    """Assign nodes to W windows of <=128 slots, balancing edge (degree) sums.
    Serpentine deal over degree-sorted nodes, then a swap repair pass.
    Returns (win_of_node, pos_of_node, max_sum)."""
    order = np.argsort(-deg, kind="stable")
    win_of = np.empty(N, dtype=np.int64)
    pos_of = np.empty(N, dtype=np.int64)
    rows = math.ceil(N / W)
    for r in range(rows):
        chunk = order[r * W:(r + 1) * W]
        cols = np.arange(chunk.shape[0])
        if r % 2 == 1:
            cols = W - 1 - cols
        win_of[chunk] = cols
        pos_of[chunk] = r
    sums = np.bincount(win_of, weights=deg, minlength=W).astype(np.int64)
    counts = np.bincount(win_of, minlength=W)

    # repair toward cap: move a minimally-sized node out of each over-cap
    # window into the emptiest window that has a free slot and room
    cap = 128 * max(1, math.ceil(sums.mean() / 128))
    if sums.max() > cap:
        members = [list(np.where(win_of == w)[0]) for w in range(W)]
        for _ in range(5000):
            hi = int(np.argmax(sums))
            if sums[hi] <= cap:
                break
            need = sums[hi] - cap
            cands = sorted(members[hi], key=lambda n: deg[n])
            cand = next((n for n in cands if deg[n] >= need), cands[-1])
            d = deg[cand]
            blocked = (counts >= 128) | (sums + d > cap)
            blocked[hi] = True
            if blocked.all():
                break
            lo = int(np.argmin(np.where(blocked, np.iinfo(np.int64).max, sums)))
            members[hi].remove(cand)
            members[lo].append(cand)
            win_of[cand] = lo
            sums[hi] -= d
            sums[lo] += d
            counts[hi] -= 1
            counts[lo] += 1
    # recompute positions within windows
    pos_of = np.zeros(N, dtype=np.int64)
    order2 = np.argsort(win_of, kind="stable")
    start = 0
    for w, c in enumerate(np.bincount(win_of, minlength=W)):
        pos_of[order2[start:start + c]] = np.arange(c)
        start += c
    return win_of, pos_of, int(sums.max())


def _prep(x, edge_attr, edge_index):
    src = np.asarray(edge_index[0], dtype=np.int64)
    dst = np.asarray(edge_index[1], dtype=np.int64)
    x = np.asarray(x, dtype=np.float32)
    edge_attr = np.asarray(edge_attr, dtype=np.float32)

    deg = np.bincount(dst, minlength=N).astype(np.int64)
    win_of, pos_of, max_sum = _balance_windows(deg)
    tpw = max(1, math.ceil(max_sum / 128))
    T = WPC * tpw            # tiles per core
    vid = win_of * 128 + pos_of

    # edge -> (core, tile, lane)
    wd = win_of[dst]
    order_e = np.argsort(wd, kind="stable")
    wds = wd[order_e]
    cnt = np.bincount(wd, minlength=W)
    starts = np.zeros(W, dtype=np.int64)
    starts[1:] = np.cumsum(cnt)[:-1]
    pos_in_w = np.arange(E, dtype=np.int64) - starts[wds]
    tile_in_w = pos_in_w >> 7
    lane = pos_in_w & 127
    core_e = wds // WPC
    flat_t = core_e * T + (wds % WPC) * tpw + tile_in_w  # [E] in sorted order

    TT_all = CORES * T
    srcs = np.zeros((TT_all, 128), dtype=np.int64)
    slots = np.zeros((TT_all, 128), dtype=np.int64)
    valid = np.zeros((TT_all, 128), dtype=bool)
    eidx = np.zeros((TT_all, 128), dtype=np.int64)
    srcs[flat_t, lane] = src[order_e]
    slots[flat_t, lane] = pos_of[dst[order_e]]
    eidx[flat_t, lane] = order_e
    valid[flat_t, lane] = True

    xbf = x.astype(ml_dtypes.bfloat16)
    eabf = edge_attr.astype(ml_dtypes.bfloat16)
    xdst_nodes = np.zeros(NV, dtype=np.int64)  # virtual slot -> node (0 pad ok)
    node_mask = np.zeros(NV, dtype=bool)
    xdst_nodes[vid] = np.arange(N)
    node_mask[vid] = True

    per_core = []
    r128 = np.arange(128)
    for c in range(CORES):
        sl = slice(c * T, (c + 1) * T)
        sc, vv, ei = srcs[sl], valid[sl], eidx[sl]
        st = slots[sl]
        blob = np.zeros((T, 128, 896), dtype=ml_dtypes.bfloat16)
        xs = xbf[sc]                       # [T,128,256]
        blob[:, :, 0:256] = np.concatenate(
            (xs[:, :, 0:128].transpose(0, 2, 1),
             xs[:, :, 128:256].transpose(0, 2, 1)), axis=2)
        # dst rows for Q: window*128 + slot -> node
        wglob = (np.arange(T) // tpw) + c * WPC
        vslot = wglob[:, None] * 128 + st
        dsts_c = xdst_nodes[vslot]
        xd = xbf[dsts_c]
        blob[:, :, 256:512] = np.concatenate(
            (xd[:, :, 0:128].transpose(0, 2, 1),
             xd[:, :, 128:256].transpose(0, 2, 1)), axis=2)
        ea = eabf[ei]
        blob[:, :, 512:768] = np.concatenate(
            (ea[:, :, 0:128].transpose(0, 2, 1),
             ea[:, :, 128:256].transpose(0, 2, 1)), axis=2)
        selb = np.zeros((T, 128, 128), dtype=ml_dtypes.bfloat16)
        tI, lI = np.nonzero(vv)
        selb[tI, lI, st[tI, lI]] = 1.0
        blob[:, :, 768:896] = selb

        # x window rows (residual), virtual order, bf16
        vsl = slice(c * WPC * 128, (c + 1) * WPC * 128)
        xwin = np.zeros((WPC * 128, DIM), dtype=ml_dtypes.bfloat16)
        m = node_mask[vsl]
        xwin[m] = xbf[xdst_nodes[vsl][m]]
        per_core.append(dict(
            blob=np.ascontiguousarray(blob.reshape(WPC, tpw, 128, 896)),
            xwin=xwin))
    return dict(per_core=per_core, tpw=tpw, vid=vid)


# ------------------------------------------------------------- phase1 kernel

def _build_phase1(tpw, split_waits=True):
    from contextlib import ExitStack
    nc = bass.Bass()
    blob_d = nc.declare_dram_parameter("blob", [WPC, tpw, 128, 896], BF16,
                                       isOutput=False)
    xwin_d = nc.declare_dram_parameter("xwin", [WPC * 128, DIM], BF16,
                                       isOutput=False)
    wkv_d = nc.declare_dram_parameter("wkv", [2, 128, 2 * DIM], BF16, isOutput=False)
    wq_d = nc.declare_dram_parameter("wq", [2, 128, DIM], BF16, isOutput=False)
    we_d = nc.declare_dram_parameter("we", [2, 128, DIM], BF16, isOutput=False)
    hpre_d = nc.declare_dram_parameter("hpre", [WPC * 128, DIM], BF16, isOutput=True)
    bns_d = nc.declare_dram_parameter("bns", [2, DIM], F32, isOutput=True)

    mul = mybir.AluOpType.mult
    add = mybir.AluOpType.add
    G = WPC * tpw

    with tile.TileContext(nc) as tc, ExitStack() as ctx:
        const = ctx.enter_context(tc.tile_pool(name="const", bufs=1))
        wkv_sb = const.tile([128, 2, 2 * DIM], BF16, tag="wkv")
        nc.sync.dma_start(wkv_sb[:], wkv_d[:].rearrange("c p f -> p c f"))
        wq_sb = const.tile([128, 2, DIM], BF16, tag="wq")
        nc.sync.dma_start(wq_sb[:], wq_d[:].rearrange("c p f -> p c f"))
        we_sb = const.tile([128, 2, DIM], BF16, tag="we")
        nc.sync.dma_start(we_sb[:], we_d[:].rearrange("c p f -> p c f"))
        ones_col = const.tile([128, 1], BF16, tag="ones_col")
        nc.vector.memset(ones_col[:], 1.0)

        # PSUM pools: kv 4, eq 2, wvz 1, bn 1 -> 8 banks
        pkv = ctx.enter_context(tc.tile_pool(name="pkv", bufs=4, space="PSUM"))
        peq = ctx.enter_context(tc.tile_pool(name="peq", bufs=2, space="PSUM"))
        pwvz = ctx.enter_context(tc.tile_pool(name="pwvz", bufs=1, space="PSUM"))
        pbn = ctx.enter_context(tc.tile_pool(name="pbn", bufs=1, space="PSUM"))
        bn_ps = pbn.tile([1, 2 * DIM], F32, tag="bn")

        p_blob = ctx.enter_context(tc.tile_pool(name="p_blob", bufs=3))
        p_m = ctx.enter_context(tc.tile_pool(name="p_m", bufs=4))
        p_small = ctx.enter_context(tc.tile_pool(name="p_small", bufs=4))
        p_h = ctx.enter_context(tc.tile_pool(name="p_h", bufs=2))
        p_xw = ctx.enter_context(tc.tile_pool(name="p_xw", bufs=2))

        def mm(out, lhsT, rhs, start, stop, **kw):
            nc.tensor.matmul(out, lhsT, rhs, start=start, stop=stop, **kw)

        # software-pipelined stages over global tile index g:
        #   B(g):    blob DMA (per window, sync queue) + kv/eq projections
        #   C(g):    eh/qg drains (scalar), m1 (vector), s2 (gpsimd)
        #   D1(g):   reduce (vector), clip (gpsimd)
        #   D2(g):   exp (scalar), msg (vector); window finalize after last D2
        #   D3(g):   seg-sum matmul (PE) -- own stage so the PE starts each
        #            iteration with a seg whose msg is already done
        state = {}

        def stage_B(g):
            w, t = divmod(g, tpw)
            if t == 0:
                bw = p_blob.tile([128, tpw, 896], BF16, tag="blob")
                nc.sync.dma_start(
                    bw[:, :, 0:512],
                    blob_d[w, :, :, 0:512].rearrange("t p f -> p t f"))
                nc.sync.dma_start(
                    bw[:, :, 512:896],
                    blob_d[w, :, :, 512:896].rearrange("t p f -> p t f"))
                xw = p_xw.tile([128, DIM], BF16, tag="xw")
                nc.sync.dma_start(xw[:], xwin_d[w * 128:(w + 1) * 128, :])
                state[("bw", w)] = bw
                state[("xw", w)] = xw
            b = state[("bw", w)][:, t, :]
            kv = pkv.tile([128, 2 * DIM], F32, tag="kv")
            mm(kv[:], b[:, 0:128], wkv_sb[:, 0, :], True, False)
            mm(kv[:], b[:, 128:256], wkv_sb[:, 1, :], False, True)
            eq = peq.tile([128, 2 * DIM], F32, tag="eq")
            mm(eq[:, 0:DIM], b[:, 512:640], we_sb[:, 0, :], True, False)
            mm(eq[:, 0:DIM], b[:, 640:768], we_sb[:, 1, :], False, True)
            mm(eq[:, DIM:2 * DIM], b[:, 256:384], wq_sb[:, 0, :], True, False)
            mm(eq[:, DIM:2 * DIM], b[:, 384:512], wq_sb[:, 1, :], False, True)
            state[("kv", g)] = kv
            state[("eq", g)] = eq

        def stage_C(g):
            kv, eq = state[("kv", g)], state.pop(("eq", g))
            eh_sb = p_m.tile([128, DIM], BF16, tag="ehsb")
            nc.scalar.copy(eh_sb[:], eq[:, 0:DIM])
            qg_sb = p_m.tile([128, DIM], BF16, tag="qgsb")
            nc.scalar.copy(qg_sb[:], eq[:, DIM:2 * DIM])
            m1 = p_m.tile([128, DIM], BF16, tag="m1")
            nc.vector.tensor_tensor(out=m1[:], in0=kv[:, 0:DIM],
                                    in1=eh_sb[:], op=mul)
            s2 = p_m.tile([128, DIM], BF16, tag="s2")
            nc.gpsimd.tensor_tensor(out=s2[:], in0=m1[:], in1=qg_sb[:], op=mul)
            state[("s2", g)] = s2

        def stage_D1(g):
            s2 = state.pop(("s2", g))
            hs = p_small.tile([128, H], F32, tag="hs")
            nc.vector.tensor_reduce(
                out=hs[:, :, None],
                in_=s2[:].rearrange("p (h d) -> p h d", d=DH),
                op=add, axis=mybir.AxisListType.X)
            hc = p_small.tile([128, H], F32, tag="hc")
            nc.gpsimd.tensor_scalar(out=hc[:], in0=hs[:], scalar1=5.0,
                                    scalar2=-5.0, op0=mybir.AluOpType.min,
                                    op1=mybir.AluOpType.max)
            state[("hc", g)] = hc

        def stage_D2(g):
            kv = state.pop(("kv", g))
            hc = state.pop(("hc", g))
            msgz = p_m.tile([128, DIM + H], BF16, tag="msgz")
            nc.scalar.activation(msgz[:, DIM:DIM + H], hc[:],
                                 mybir.ActivationFunctionType.Exp)
            nc.vector.tensor_tensor(
                out=msgz[:, 0:DIM].rearrange("p (h d) -> p h d", d=DH),
                in0=kv[:, DIM:2 * DIM].rearrange("p (h d) -> p h d", d=DH),
                in1=msgz[:, DIM:DIM + H, None].to_broadcast([128, H, DH]),
                op=mul)
            state[("msgz", g)] = msgz

        def stage_D3(g):
            w, t = divmod(g, tpw)
            msgz = state.pop(("msgz", g))
            if t == 0:
                state[("wvz", w)] = pwvz.tile([128, DIM + H], F32, tag="wvz",
                                              name="wvz")
            wvz = state[("wvz", w)]
            b = state[("bw", w)][:, t, :]
            mm(wvz[:], b[:, 768:896], msgz[:], t == 0, t == tpw - 1)
            if t == tpw - 1:
                finalize(w)

        def finalize(w):
            wvz = state.pop(("wvz", w))
            state.pop(("bw", w))
            xw = state.pop(("xw", w))
            zr = p_small.tile([128, H], F32, tag="zr")
            nc.vector.tensor_scalar(out=zr[:], in0=wvz[:, DIM:DIM + H],
                                    scalar1=EPS_Z, scalar2=None, op0=add)
            nc.vector.reciprocal(zr[:], zr[:])
            h = p_h.tile([128, DIM], BF16, tag="h")
            nc.vector.tensor_tensor(
                out=h[:].rearrange("p (h d) -> p h d", d=DH),
                in0=wvz[:, 0:DIM].rearrange("p (h d) -> p h d", d=DH),
                in1=zr[:, :, None].to_broadcast([128, H, DH]), op=mul)
            nc.gpsimd.tensor_tensor(out=h[:], in0=h[:], in1=xw[:], op=add)
            nc.gpsimd.dma_start(hpre_d[w * 128:(w + 1) * 128, :], h[:])
            sq = p_h.tile([128, DIM], BF16, tag="sq")
            nc.gpsimd.tensor_tensor(out=sq[:], in0=h[:], in1=h[:], op=mul)
            state[("h", w)] = h
            state[("sq", w)] = sq

        def bn_mms(w):
            # deferred one window so the PE never waits on fresh h/sq
            h = state.pop(("h", w))
            sq = state.pop(("sq", w))
            mm(bn_ps[:, 0:DIM], ones_col[:], h[:], w == 0, w == WPC - 1,
               skip_group_check=True)
            mm(bn_ps[:, DIM:2 * DIM], ones_col[:], sq[:], w == 0, w == WPC - 1,
               skip_group_check=True)

        for i in range(G + 4):
            if i >= 4:
                stage_D3(i - 4)
            if i < G:
                stage_B(i)
            if i >= 4 and (i - 4) % tpw == tpw - 1 and (i - 4) >= tpw:
                bn_mms((i - 4) // tpw - 1)
            if 1 <= i <= G:
                stage_C(i - 1)
            if 2 <= i <= G + 1:
                stage_D1(i - 2)
            if 3 <= i <= G + 2:
                stage_D2(i - 3)
        bn_mms(WPC - 1)

        bn_sb = p_small.tile([1, 2 * DIM], F32, tag="bnsb")
        nc.vector.tensor_copy(bn_sb[:], bn_ps[:])
        nc.scalar.dma_start(bns_d[:].rearrange("a b -> (a b)")[None, :], bn_sb[:])

    return _split_excess_waits(nc) if split_waits else nc


def _split_excess_waits(nc, max_waits=1):
    """Most HW-decoded opcodes carry only ~1 sync wait; move the excess onto
    preceding same-engine NoOps, which use the sequencer wait table."""
    k = 0
    skip = {"InstNoOp"}
    for f in nc.m.functions:
        for b in f.blocks:
            new = []
            for inst in b.instructions:
                si = inst.sync_info
                if (type(inst).__name__ not in skip and si is not None
                        and si.on_wait and len(si.on_wait) > max_waits):
                    extra = si.on_wait[:-max_waits]
                    for wt in extra:
                        nop = mybir.InstNoOp(name=f"I-wsplit{k}", ins=[], outs=[])
                        k += 1
                        nop.engine = inst.engine
                        nop.bass_nofuse = True
                        nop.sync_info = mybir.SyncInfo(on_wait=[wt], on_update=[])
                        new.append(nop)
                    inst.sync_info = mybir.SyncInfo(
                        on_wait=si.on_wait[-max_waits:], on_update=si.on_update)
                new.append(inst)
            b.instructions = new
    return nc


# ------------------------------------------------------------- phase2 kernel

def _build_phase2():
    from contextlib import ExitStack
    nc = bass.Bass()
    win = WPC
    hpre_d = nc.declare_dram_parameter("hpre", [win * 128, DIM], BF16, isOutput=False)
    scale_d = nc.declare_dram_parameter("scale_rep", [128, DIM], F32, isOutput=False)
    shift_d = nc.declare_dram_parameter("shift_rep", [128, DIM], F32, isOutput=False)
    hout_d = nc.declare_dram_parameter("hout", [win * 128, DIM], F32, isOutput=True)
    mul = mybir.AluOpType.mult
    add = mybir.AluOpType.add
    with tile.TileContext(nc) as tc, ExitStack() as ctx:
        const = ctx.enter_context(tc.tile_pool(name="const", bufs=1))
        scale = const.tile([128, DIM], F32, tag="scale")
        nc.sync.dma_start(scale[:], scale_d[:])
        shift = const.tile([128, DIM], F32, tag="shift")
        nc.sync.dma_start(shift[:], shift_d[:])
        pool = ctx.enter_context(tc.tile_pool(name="ht", bufs=4))
        hp = hpre_d[:].rearrange("(b w p) f -> b p w f", p=128, w=2)
        ho = hout_d[:].rearrange("(b w p) f -> b p w f", p=128, w=2)
        nblk = hp.shape[0]
        for b in range(nblk):
            ht = pool.tile([128, 2, DIM], BF16, tag="ht")
            nc.sync.dma_start(ht[:], hp[b])
            hf = pool.tile([128, 2, DIM], F32, tag="hf")
            nc.vector.tensor_tensor(
                out=hf[:], in0=ht[:],
                in1=scale[:, None, :].to_broadcast([128, 2, DIM]), op=mul)
            nc.vector.tensor_tensor(
                out=hf[:], in0=hf[:],
                in1=shift[:, None, :].to_broadcast([128, 2, DIM]), op=add)
            nc.scalar.dma_start(ho[b], hf[:])
    return _split_excess_waits(nc)


# ------------------------------------------------------------------- runner

def _install_ntff_hook():
    import sys, types
    if "antenv.axon_hooks" in sys.modules:
        return True
    try:
        import antenv
        from trn_agent_boot.trn_boot import _ntff_profile_via_ctypes
        mod = types.ModuleType("antenv.axon_hooks")
        mod._hook = _ntff_profile_via_ctypes("/opt/axon/libaxon_pjrt.so")
        mod.set_axon_ntff_profile_hook = lambda h: setattr(mod, "_hook", h)
        mod.get_axon_ntff_profile_hook = lambda: mod._hook
        sys.modules["antenv.axon_hooks"] = mod
        antenv.axon_hooks = mod
        return mod._hook is not None
    except Exception:
        return False


_CACHE = {}


def _get_phase1(tpw):
    key = ("p1", tpw)
    if key not in _CACHE:
        _CACHE[key] = _build_phase1(tpw)
    return _CACHE[key]


def _get_phase2():
    key = ("p2",)
    if key not in _CACHE:
        _CACHE[key] = _build_phase2()
    return _CACHE[key]


def run_pipeline(x, edge_attr, WQ, WK, WE, WV, gamma, beta, edge_index,
                 timed=False):
    prep = _prep(x, edge_attr, edge_index)
    tpw = prep["tpw"]
    scale_inv = np.float32(1.0 / math.sqrt(DH))

    def cast(a):
        return np.ascontiguousarray(a).astype(ml_dtypes.bfloat16)

    wq = cast((np.asarray(WQ, np.float32) * scale_inv).reshape(2, 128, DIM))
    we = cast(np.asarray(WE, np.float32).reshape(2, 128, DIM))
    wkv = cast(np.concatenate(
        [np.asarray(WK, np.float32).reshape(2, 128, DIM),
         np.asarray(WV, np.float32).reshape(2, 128, DIM)], axis=2))

    nc1 = _get_phase1(tpw)
    in_maps = []
    for c in range(CORES):
        pc = prep["per_core"][c]
        in_maps.append(dict(blob=pc["blob"], xwin=pc["xwin"],
                            wkv=wkv, wq=wq, we=we))

    trace = timed and _install_ntff_hook()
    r1 = run_bass_kernel_spmd(nc1, in_maps, list(range(CORES)), trace=trace)
    results1 = r1.results
    t1 = r1.exec_time_ns
    hpres = [results1[c]["hpre"] for c in range(CORES)]
    bns = sum(results1[c]["bns"].astype(np.float64) for c in range(CORES))
    mean = bns[0] / N
    var = bns[1] / N - mean * mean
    scale = (np.asarray(gamma, np.float64) / np.sqrt(var + EPS_BN))
    shift = np.asarray(beta, np.float64) - mean * scale

    nc2 = _get_phase2()
    scale_rep = np.ascontiguousarray(np.tile(scale.astype(np.float32), (128, 1)))
    shift_rep = np.ascontiguousarray(np.tile(shift.astype(np.float32), (128, 1)))
    in_maps2 = [dict(hpre=hpres[c], scale_rep=scale_rep, shift_rep=shift_rep)
                for c in range(CORES)]
    r2 = run_bass_kernel_spmd(nc2, in_maps2, list(range(CORES)), trace=trace)
    t2 = r2.exec_time_ns
    hv = np.concatenate([r2.results[c]["hout"] for c in range(CORES)])
    out = hv[prep["vid"]]
    info = dict(t1=t1, t2=t2, tpw=tpw)
    return np.ascontiguousarray(out.astype(np.float32)), info


def kernel(x, edge_attr, WQ, WK, WE, WV, gamma, beta, edge_index):
    out, _ = run_pipeline(x, edge_attr, WQ, WK, WE, WV, gamma, beta, edge_index)
    return out

